# revision 1
# baseline (speedup 1.0000x reference)
"""TRN2 Bass kernel for nn_GTLayer (ELL sparse attention, N=50000, K=16).

Sharding: nodes split contiguously across 8 NeuronCores (6250/core, padded
to 6272). Per core: embedding-gather -> h, PE matmuls -> q/k/v (k|v rows
interleaved), on-device AllGather of kv, then per 128-node tile 16
indirect-DMA neighbor-row gathers + DVE attention. Masking uses
t=(s+120)*mask, exp(0.25*t-30): masked lanes get exp(-30)~=1e-13, and a
fully-masked row degrades to the uniform average exactly like jax softmax.
"""
import numpy as np

import concourse.bass as bass
import concourse.mybir as mybir
import concourse.tile as tile
from concourse.masks import make_identity
from concourse.vector_clock import ScopedClock

F32 = mybir.dt.float32
I32 = mybir.dt.int32
U8 = mybir.dt.uint8
F16 = mybir.dt.float16
AX = mybir.AxisListType
ALU = mybir.AluOpType
AF = mybir.ActivationFunctionType

N_FEATS, VOCAB, HID, NH, HD, K = 9, 119, 128, 8, 16, 16
VFLAT = N_FEATS * VOCAB
P = 128
NCORES = 8
NRC = 6250          # real nodes per core
NPC = 6272          # padded nodes per core (49 x 128)

# ---------------------------------------------------------------- walrus fixes
# This walrus build rejects >1 sync-wait command per instruction. Two fixes:
# (1) TileContext tail drain: emit waits as single-wait nops.
# (2) General: split multi-wait instructions in the serialized BIR JSON by
#     inserting single-wait NoOps immediately before them (order preserved).


def _patched_drain_and_barrier(self, tick_clock, wait_clock):
    nc = self.nc
    probe = nc.sync.nop(nofuse=True)
    wait_clock.add_sem_waits(probe.ins, ScopedClock({None: tick_clock.global_clock}))
    waits = list(probe.ins.sync_info.on_wait or []) if probe.ins.sync_info else []
    if probe.ins.sync_info:
        probe.ins.sync_info.on_wait = waits[:1]
    for w in waits[1:]:
        n2 = nc.sync.nop(nofuse=True)
        if n2.ins.sync_info is None:
            n2.ins.sync_info = mybir.SyncInfo(on_update=[], on_wait=[w])
        else:
            n2.ins.sync_info.on_wait = [w]
    nc.sync.drain()
    nc.all_engine_barrier()
    assert self.sems is not None
    popped = nc._tile_sem_poison_stack.pop()
    assert popped is self._sem_poison
    nc.clear_and_free_semaphores(list(self.sems.allocated().values()))
    nc.all_engine_barrier()


tile.TileContext._drain_and_barrier = _patched_drain_and_barrier


def _split_waits_json(bir_bytes):
    import orjson
    m = orjson.loads(bir_bytes)
    n = 0
    for fn in m["functions"]:
        for blk in fn["blocks"]:
            new = []
            for ins in blk["instructions"]:
                si = ins.get("sync_info")
                waits = (si or {}).get("on_wait") or []
                if len(waits) > 1:
                    for w in waits[:-1]:
                        n += 1
                        new.append({
                            "debug": ins.get("debug", 0),
                            "engine": ins["engine"],
                            "ins": [], "name": f"I-wfix-{n}",
                            "opcode": "NoOp", "outs": [],
                            "sync_info": {"on_update": [], "on_wait": [w]},
                        })
                    si["on_wait"] = waits[-1:]
                new.append(ins)
            blk["instructions"] = new
    return orjson.dumps(m), n


import concourse.bass2jax as _b2j

_orig_cbk = _b2j.compile_bir_kernel


def _patched_cbk(ant_bir_str, *a, **kw):
    fixed, n = _split_waits_json(ant_bir_str)
    return _orig_cbk(fixed, *a, **kw)


_b2j.compile_bir_kernel = _patched_cbk

# ---------------------------------------------------------------- device code


def build(nc, npad_core=NPC, ncores=NCORES):
    T = npad_core // P
    ntot = npad_core * ncores

    xc = nc.dram_tensor("xc", [npad_core, N_FEATS], I32, kind="ExternalInput")
    nb = nc.dram_tensor("nb", [npad_core, K], I32, kind="ExternalInput")
    mk = nc.dram_tensor("mk", [npad_core, K], U8, kind="ExternalInput")
    emb = nc.dram_tensor("emb", [VFLAT, HID], F32, kind="ExternalInput")
    wq = nc.dram_tensor("wq", [HID, HID], F32, kind="ExternalInput")
    wk = nc.dram_tensor("wk", [HID, HID], F32, kind="ExternalInput")
    wv = nc.dram_tensor("wv", [HID, HID], F32, kind="ExternalInput")
    bq = nc.dram_tensor("bq", [HID, 1], F32, kind="ExternalInput")
    bk = nc.dram_tensor("bk", [HID, 1], F32, kind="ExternalInput")
    bv = nc.dram_tensor("bv", [HID, 1], F32, kind="ExternalInput")
    out = nc.dram_tensor("out", [npad_core, HID], F32, kind="ExternalOutput")

    with tile.TileContext(nc) as tc:
        with (
            tc.tile_pool(name="const", bufs=1) as cp,
            tc.tile_pool(name="resident", bufs=1) as rp,
            tc.tile_pool(name="work", bufs=3) as wp,
            tc.tile_pool(name="gath", bufs=3) as gp,
            tc.tile_pool(name="psum", bufs=2, space="PSUM") as pp,
            tc.tile_pool(name="dram", bufs=1, space="DRAM") as dp,
        ):
            ident = cp.tile([P, P], F32, name="ident")
            make_identity(nc, ident[:])
            negc = cp.tile([P, 1], F32, name="negc")
            nc.gpsimd.memset(negc[:], -30.0)
            w_q = cp.tile([HID, HID], F32, name="w_q")
            w_k = cp.tile([HID, HID], F32, name="w_k")
            w_v = cp.tile([HID, HID], F32, name="w_v")
            b_q = cp.tile([HID, 1], F32, name="b_q")
            b_k = cp.tile([HID, 1], F32, name="b_k")
            b_v = cp.tile([HID, 1], F32, name="b_v")
            for t_, d_ in ((w_q, wq), (w_k, wk), (w_v, wv),
                           (b_q, bq), (b_k, bk), (b_v, bv)):
                nc.sync.dma_start(out=t_[:], in_=d_[:])

            q_all = rp.tile([P, T * HID], F16, name="q_all")
            idx_all = rp.tile([P, T * K], I32, name="idx_all")
            msk_all = rp.tile([P, T * K], F32, name="msk_all")

            kv_shard = dp.tile([npad_core, 2 * HID], F16, name="kv_shard")
            kv_full = dp.tile([ntot, 2 * HID], F16, name="kv_full",
                              addr_space="Shared")

            # phase 1: h -> q,k,v
            for t in range(T):
                r0 = t * P
                xt = wp.tile([P, N_FEATS], I32, name="xt")
                nc.sync.dma_start(out=xt[:], in_=xc[r0:r0 + P, :])
                mt8 = wp.tile([P, K], U8, name="mt8")
                nc.sync.dma_start(out=mt8[:], in_=mk[r0:r0 + P, :])
                nc.vector.tensor_copy(out=msk_all[:, t * K:(t + 1) * K], in_=mt8[:])
                nc.sync.dma_start(out=idx_all[:, t * K:(t + 1) * K],
                                  in_=nb[r0:r0 + P, :])

                et = wp.tile([P, N_FEATS * HID], F32, name="et")
                for f in range(N_FEATS):
                    nc.gpsimd.indirect_dma_start(
                        out=et[:, f * HID:(f + 1) * HID], out_offset=None,
                        in_=emb[:],
                        in_offset=bass.IndirectOffsetOnAxis(
                            ap=xt[:, f:f + 1], axis=0))
                ht = wp.tile([P, HID], F32, name="ht")
                nc.vector.tensor_reduce(
                    out=ht[:],
                    in_=et[:].rearrange("p (f c) -> p c f", f=N_FEATS),
                    axis=AX.X, op=ALU.add)

                hT_p = pp.tile([P, P], F32, name="hT_p", space="PSUM")
                nc.tensor.transpose(out=hT_p[:], in_=ht[:], identity=ident[:])
                hT = wp.tile([P, P], F32, name="hT")
                nc.scalar.copy(out=hT[:], in_=hT_p[:])

                for wmat, bias, dst in (
                        (w_q, b_q, "q"), (w_k, b_k, "k"), (w_v, b_v, "v")):
                    yT_p = pp.tile([P, P], F32, name="yT_p", space="PSUM")
                    nc.tensor.matmul(out=yT_p[:], lhsT=wmat[:], rhs=hT[:],
                                     start=True, stop=True)
                    yT = wp.tile([P, P], F32, name="yT")
                    nc.vector.tensor_scalar_add(out=yT[:], in0=yT_p[:],
                                                scalar1=bias[:])
                    y_p = pp.tile([P, P], F32, name="y_p", space="PSUM")
                    nc.tensor.transpose(out=y_p[:], in_=yT[:], identity=ident[:])
                    if dst == "q":
                        nc.scalar.copy(out=q_all[:, t * HID:(t + 1) * HID],
                                       in_=y_p[:])
                    elif dst == "k":
                        kvt = wp.tile([P, 2 * HID], F16, name="kvt")
                        nc.scalar.copy(out=kvt[:, 0:HID], in_=y_p[:])
                    else:
                        nc.scalar.copy(out=kvt[:, HID:2 * HID], in_=y_p[:])
                        nc.sync.dma_start(out=kv_shard[r0:r0 + P, :], in_=kvt[:])

            # phase 2: allgather kv across the 8 cores
            nc.gpsimd.collective_compute(
                "AllGather", ALU.bypass,
                replica_groups=[list(range(ncores))],
                ins=[kv_shard[:]], outs=[kv_full[:]])

            # phase 3: neighbor gather + attention
            lp = nc.allow_low_precision(reason="fp16 attention scores")
            lp.__enter__()
            for t in range(T):
                r0 = t * P
                knvn = gp.tile([P, K * 2 * HID], F16, name="knvn")
                for j in range(K):
                    nc.gpsimd.indirect_dma_start(
                        out=knvn[:, j * 2 * HID:(j + 1) * 2 * HID],
                        out_offset=None, in_=kv_full[:],
                        in_offset=bass.IndirectOffsetOnAxis(
                            ap=idx_all[:, t * K + j:t * K + j + 1], axis=0))
                kn = knvn[:].rearrange("p (j c) -> p j c", j=K)[:, :, 0:HID]
                vn = knvn[:].rearrange("p (j c) -> p j c", j=K)[:, :, HID:2 * HID]

                qb = q_all[:, t * HID:(t + 1) * HID] \
                    .rearrange("p (a c) -> p a c", a=1).to_broadcast([P, K, HID])
                prod = wp.tile([P, K * HID], F32, name="prod")
                nc.vector.tensor_tensor(
                    out=prod[:].rearrange("p (j c) -> p j c", j=K),
                    in0=kn, in1=qb, op=ALU.mult)

                s = wp.tile([P, K * NH], F32, name="s")
                nc.vector.tensor_reduce(
                    out=s[:],
                    in_=prod[:].rearrange("p (j h d) -> p j h d", j=K, h=NH),
                    axis=AX.X, op=ALU.add)

                mb = msk_all[:, t * K:(t + 1) * K] \
                    .rearrange("p (j a) -> p j a", a=1).to_broadcast([P, K, NH])
                tt = wp.tile([P, K * NH], F32, name="tt")
                nc.vector.scalar_tensor_tensor(
                    out=tt[:].rearrange("p (j h) -> p j h", j=K),
                    in0=s[:].rearrange("p (j h) -> p j h", j=K),
                    scalar=120.0, in1=mb, op0=ALU.add, op1=ALU.mult)

                e = wp.tile([P, K * NH], F32, name="e")
                nc.scalar.activation(out=e[:], in_=tt[:], func=AF.Exp,
                                     bias=negc[:], scale=0.25)

                z = wp.tile([P, NH], F32, name="z")
                nc.vector.tensor_reduce(
                    out=z[:], in_=e[:].rearrange("p (j h) -> p h j", j=K),
                    axis=AX.X, op=ALU.add)
                zr = wp.tile([P, NH], F32, name="zr")
                nc.vector.reciprocal(out=zr[:], in_=z[:])


                at = wp.tile([P, K * NH], F32, name="at")
                nc.vector.tensor_tensor(
                    out=at[:].rearrange("p (j h) -> p j h", j=K),
                    in0=e[:].rearrange("p (j h) -> p j h", j=K),
                    in1=zr[:].rearrange("p (a h) -> p a h", a=1)
                        .to_broadcast([P, K, NH]),
                    op=ALU.mult)

                prod2 = wp.tile([P, K * HID], F32, name="prod2")
                nc.vector.tensor_tensor(
                    out=prod2[:].rearrange("p (j h d) -> p j h d", j=K, h=NH),
                    in0=vn.rearrange("p j (h d) -> p j h d", h=NH),
                    in1=at[:].rearrange("p (j h) -> p j h", j=K)
                        .rearrange("p j (h a) -> p j h a", a=1)
                        .to_broadcast([P, K, NH, HD]),
                    op=ALU.mult)

                o = wp.tile([P, HID], F32, name="o")
                nc.vector.tensor_reduce(
                    out=o[:],
                    in_=prod2[:].rearrange("p (j c) -> p c j", j=K),
                    axis=AX.X, op=ALU.add)
                nc.sync.dma_start(out=out[r0:r0 + P, :], in_=o[:])
            lp.__exit__(None, None, None)
    return nc


# ---------------------------------------------------------------- host side


def _prep(X, nbr_idx, nbr_mask, atom_emb, Wq, bq, Wk, bk, Wv, bv):
    offs = (np.arange(N_FEATS, dtype=np.int64) * VOCAB)[None, :]
    xc_full = (np.asarray(X).astype(np.int64) + offs).astype(np.int32)
    g = np.asarray(nbr_idx).astype(np.int64)
    remap = ((g // NRC) * NPC + (g % NRC)).astype(np.int32)
    mask = np.asarray(nbr_mask).astype(np.uint8)
    emb_flat = np.ascontiguousarray(
        np.asarray(atom_emb, dtype=np.float32).reshape(VFLAT, HID))
    maps = []
    for r in range(NCORES):
        lo, hi = r * NRC, (r + 1) * NRC
        xcp = np.zeros((NPC, N_FEATS), np.int32)
        xcp[:NRC] = xc_full[lo:hi]
        nbp = np.zeros((NPC, K), np.int32)
        nbp[:NRC] = remap[lo:hi]
        mkp = np.zeros((NPC, K), np.uint8)
        mkp[:NRC] = mask[lo:hi]
        maps.append({
            "xc": xcp, "nb": nbp, "mk": mkp, "emb": emb_flat,
            "wq": np.ascontiguousarray(np.asarray(Wq, np.float32)),
            "wk": np.ascontiguousarray(np.asarray(Wk, np.float32)),
            "wv": np.ascontiguousarray(np.asarray(Wv, np.float32)),
            "bq": np.asarray(bq, np.float32).reshape(HID, 1),
            "bk": np.asarray(bk, np.float32).reshape(HID, 1),
            "bv": np.asarray(bv, np.float32).reshape(HID, 1),
        })
    return maps


_CACHE = {}


def run_on_device(maps, trace=False):
    from concourse.bass_utils import run_bass_kernel_spmd
    if "nc" not in _CACHE:
        nc = bass.Bass()
        build(nc)
        _CACHE["nc"] = nc
    return run_bass_kernel_spmd(_CACHE["nc"], maps, list(range(NCORES)),
                                trace=trace)


def kernel(X, nbr_idx, nbr_mask, atom_emb, Wq, bq, Wk, bk, Wv, bv):
    maps = _prep(X, nbr_idx, nbr_mask, atom_emb, Wq, bq, Wk, bk, Wv, bv)
    res = run_on_device(maps)
    return np.concatenate([r["out"][:NRC] for r in res.results], axis=0)



# revision 6
# speedup vs baseline: 1.5662x; 1.5662x over previous
"""TRN2 Bass kernel for nn_GTLayer (ELL sparse attention, N=50000, K=16).

Sharding: nodes split contiguously across 8 NeuronCores (6250/core, padded
to 6272 = 49 x 128). Per core, three phases:
  1. embedding gather (host-packed feature-PAIR tables -> 5 rows/node in one
     batched indirect DMA) -> h -> k,v projections -> kv_shard rows (f16).
  2. AllGather kv across the 8 cores; q projections + index/mask loads are
     scheduled under the collective.
  3. per 128-node tile: ONE batched indirect DMA gathers all 16 neighbor
     kv rows; attention in f16 on DVE (products + binary add-trees instead
     of TensorReduce), exp on the Act engine broadcast over head_dim, final
     out = o / z via tensor-tensor divide.
Masking: tt=(s+36)*mask, e=exp(0.25*tt-9); masked lanes get exp(-9)~1.2e-4
(f16-normal), and a fully-masked row degrades to the uniform average exactly
like jax softmax.
"""
import numpy as np

import concourse.bass as bass
import concourse.mybir as mybir
import concourse.tile as tile
from concourse.masks import make_identity
from concourse.vector_clock import ScopedClock

F32 = mybir.dt.float32
I32 = mybir.dt.int32
U8 = mybir.dt.uint8
F16 = mybir.dt.float16
AX = mybir.AxisListType
ALU = mybir.AluOpType
AF = mybir.ActivationFunctionType

N_FEATS, VOCAB, HID, NH, HD, K = 9, 119, 128, 8, 16, 16
P = 128
NCORES = 8
NRC = 6250          # real nodes per core
NPC = 6272          # padded nodes per core (49 x 128)
NPAIR = VOCAB * VOCAB
# packed embedding table: 4 pair tables + 1 single table, 128-wide f16 rows
EMB_ROWS = 4 * NPAIR + VOCAB

# ---------------------------------------------------------------- walrus fixes
# This walrus build rejects >1 sync-wait command per instruction. Two fixes:
# (1) TileContext tail drain: emit waits as single-wait nops.
# (2) General: split multi-wait instructions in the serialized BIR JSON by
#     inserting single-wait NoOps immediately before them (order preserved).


def _patched_drain_and_barrier(self, tick_clock, wait_clock):
    nc = self.nc
    probe = nc.sync.nop(nofuse=True)
    wait_clock.add_sem_waits(probe.ins, ScopedClock({None: tick_clock.global_clock}))
    waits = list(probe.ins.sync_info.on_wait or []) if probe.ins.sync_info else []
    if probe.ins.sync_info:
        probe.ins.sync_info.on_wait = waits[:1]
    for w in waits[1:]:
        n2 = nc.sync.nop(nofuse=True)
        if n2.ins.sync_info is None:
            n2.ins.sync_info = mybir.SyncInfo(on_update=[], on_wait=[w])
        else:
            n2.ins.sync_info.on_wait = [w]
    nc.sync.drain()
    nc.all_engine_barrier()
    assert self.sems is not None
    popped = nc._tile_sem_poison_stack.pop()
    assert popped is self._sem_poison
    nc.clear_and_free_semaphores(list(self.sems.allocated().values()))
    nc.all_engine_barrier()


tile.TileContext._drain_and_barrier = _patched_drain_and_barrier


def _split_waits_json(bir_bytes):
    import orjson
    m = orjson.loads(bir_bytes)
    n = 0
    for fn in m["functions"]:
        for blk in fn["blocks"]:
            new = []
            for ins in blk["instructions"]:
                si = ins.get("sync_info")
                waits = (si or {}).get("on_wait") or []
                if len(waits) > 1:
                    for w in waits[:-1]:
                        n += 1
                        new.append({
                            "debug": ins.get("debug", 0),
                            "engine": ins["engine"],
                            "ins": [], "name": f"I-wfix-{n}",
                            "opcode": "NoOp", "outs": [],
                            "sync_info": {"on_update": [], "on_wait": [w]},
                        })
                    si["on_wait"] = waits[-1:]
                new.append(ins)
            blk["instructions"] = new
    return orjson.dumps(m), n


import concourse.bass2jax as _b2j

_orig_cbk = _b2j.compile_bir_kernel


def _patched_cbk(ant_bir_str, *a, **kw):
    fixed, n = _split_waits_json(ant_bir_str)
    return _orig_cbk(fixed, *a, **kw)


_b2j.compile_bir_kernel = _patched_cbk

# ---------------------------------------------------------------- device code


def build(nc, npad_core=NPC, ncores=NCORES):
    T = npad_core // P
    ntot = npad_core * ncores

    xc = nc.dram_tensor("xc", [npad_core, 5], I32, kind="ExternalInput")
    nb = nc.dram_tensor("nb", [npad_core, K], I32, kind="ExternalInput")
    mk = nc.dram_tensor("mk", [npad_core, K], U8, kind="ExternalInput")
    emb = nc.dram_tensor("emb", [EMB_ROWS, HID], F16, kind="ExternalInput")
    wq = nc.dram_tensor("wq", [HID, HID], F32, kind="ExternalInput")
    wk = nc.dram_tensor("wk", [HID, HID], F32, kind="ExternalInput")
    wv = nc.dram_tensor("wv", [HID, HID], F32, kind="ExternalInput")
    bq = nc.dram_tensor("bq", [HID, 1], F32, kind="ExternalInput")
    bk = nc.dram_tensor("bk", [HID, 1], F32, kind="ExternalInput")
    bv = nc.dram_tensor("bv", [HID, 1], F32, kind="ExternalInput")
    out = nc.dram_tensor("out", [npad_core, HID], F32, kind="ExternalOutput")

    lp = nc.allow_low_precision(reason="f16 attention pipeline")
    lp.__enter__()
    with tile.TileContext(nc) as tc:
        with (
            tc.tile_pool(name="const", bufs=1) as cp,
            tc.tile_pool(name="resident", bufs=1) as rp,
            tc.tile_pool(name="work", bufs=3) as wp,
            tc.tile_pool(name="gath", bufs=3) as gp,
            tc.tile_pool(name="psum", bufs=2, space="PSUM") as pp,
            tc.tile_pool(name="dram", bufs=1, space="DRAM") as dp,
        ):
            ident = cp.tile([P, P], F32, name="ident")
            make_identity(nc, ident[:])
            negq = cp.tile([P, 1], F32, name="negq")
            nc.gpsimd.memset(negq[:], -9.0)
            w_q = cp.tile([HID, HID], F32, name="w_q")
            w_k = cp.tile([HID, HID], F32, name="w_k")
            w_v = cp.tile([HID, HID], F32, name="w_v")
            b_q = cp.tile([HID, 1], F32, name="b_q")
            b_k = cp.tile([HID, 1], F32, name="b_k")
            b_v = cp.tile([HID, 1], F32, name="b_v")
            for t_, d_ in ((w_q, wq), (w_k, wk), (w_v, wv),
                           (b_q, bq), (b_k, bk), (b_v, bv)):
                nc.sync.dma_start(out=t_[:], in_=d_[:])

            xt_all = rp.tile([P, T * 5], I32, name="xt_all")
            nc.sync.dma_start(
                out=xt_all[:].rearrange("p (t f) -> p t f", t=T),
                in_=xc[:].rearrange("(t p) f -> p t f", p=P))
            idx_all = rp.tile([P, T * K], I32, name="idx_all")
            msk_all = rp.tile([P, T * K], F32, name="msk_all")
            q_all = rp.tile([P, T * HID], F16, name="q_all")
            hT_all = rp.tile([P, T * HID], F32, name="hT_all")

            kv_shard = dp.tile([npad_core, 2 * HID], F16, name="kv_shard")
            kv_full = dp.tile([ntot, 2 * HID], F16, name="kv_full",
                              addr_space="Shared")

            # ---------------- phase 1: h -> k,v (q deferred to phase 2)
            for t in range(T):
                r0 = t * P
                et = gp.tile([P, 5 * HID], F16, name="et")
                nc.gpsimd.indirect_dma_start(
                    out=et[:], out_offset=None, in_=emb[:],
                    in_offset=bass.IndirectOffsetOnAxis(
                        ap=xt_all[:, t * 5:(t + 1) * 5], axis=0))
                # tree-sum the 5 row-chunks: (c01+c45, c23+c67), then fold
                eL1 = wp.tile([P, 2 * HID], F16, name="eL1")
                nc.vector.tensor_tensor(
                    out=eL1[:], in0=et[:, 0:2 * HID],
                    in1=et[:, 2 * HID:4 * HID], op=ALU.add)
                eL2 = wp.tile([P, HID], F16, name="eL2")
                nc.vector.tensor_tensor(
                    out=eL2[:], in0=eL1[:, 0:HID], in1=eL1[:, HID:2 * HID],
                    op=ALU.add)
                ht = wp.tile([P, HID], F32, name="ht")
                nc.vector.tensor_tensor(
                    out=ht[:], in0=eL2[:], in1=et[:, 4 * HID:5 * HID],
                    op=ALU.add)

                hT_p = pp.tile([P, P], F32, name="hT_p", space="PSUM")
                nc.tensor.transpose(out=hT_p[:], in_=ht[:], identity=ident[:])
                hT = hT_all[:, t * HID:(t + 1) * HID]
                nc.vector.tensor_copy(out=hT, in_=hT_p[:])

                kvt = wp.tile([P, 2 * HID], F16, name="kvt")
                for wmat, bias, col in ((w_k, b_k, 0), (w_v, b_v, HID)):
                    yT_p = pp.tile([P, P], F32, name="yT_p", space="PSUM")
                    nc.tensor.matmul(out=yT_p[:], lhsT=wmat[:], rhs=hT,
                                     start=True, stop=True)
                    yT = wp.tile([P, P], F32, name="yT")
                    nc.vector.tensor_scalar_add(out=yT[:], in0=yT_p[:],
                                                scalar1=bias[:])
                    y_p = pp.tile([P, P], F32, name="y_p", space="PSUM")
                    nc.tensor.transpose(out=y_p[:], in_=yT[:], identity=ident[:])
                    nc.scalar.copy(out=kvt[:, col:col + HID], in_=y_p[:])
                nc.sync.dma_start(out=kv_shard[r0:r0 + P, :], in_=kvt[:])

            # ---------------- phase 2: allgather kv; q + loads run under it
            nc.gpsimd.collective_compute(
                "AllGather", ALU.bypass,
                replica_groups=[list(range(ncores))],
                ins=[kv_shard[:]], outs=[kv_full[:]])

            nc.sync.dma_start(
                out=idx_all[:].rearrange("p (t k) -> p t k", t=T),
                in_=nb[:].rearrange("(t p) k -> p t k", p=P))
            mt8 = rp.tile([P, T * K], U8, name="mt8")
            nc.sync.dma_start(
                out=mt8[:].rearrange("p (t k) -> p t k", t=T),
                in_=mk[:].rearrange("(t p) k -> p t k", p=P))
            nc.vector.tensor_copy(out=msk_all[:], in_=mt8[:])

            for t in range(T):
                qT_p = pp.tile([P, P], F32, name="yT_p", space="PSUM")
                nc.tensor.matmul(out=qT_p[:], lhsT=w_q[:],
                                 rhs=hT_all[:, t * HID:(t + 1) * HID],
                                 start=True, stop=True)
                qT = wp.tile([P, P], F32, name="yT")
                nc.vector.tensor_scalar_add(out=qT[:], in0=qT_p[:],
                                            scalar1=b_q[:])
                q_p = pp.tile([P, P], F32, name="y_p", space="PSUM")
                nc.tensor.transpose(out=q_p[:], in_=qT[:], identity=ident[:])
                nc.scalar.copy(out=q_all[:, t * HID:(t + 1) * HID], in_=q_p[:])

            # ---------------- phase 3: batched neighbor gather + attention
            def gather(t):
                knvn = gp.tile([P, K * 2 * HID], F16, name="knvn")
                nc.gpsimd.indirect_dma_start(
                    out=knvn[:], out_offset=None, in_=kv_full[:],
                    in_offset=bass.IndirectOffsetOnAxis(
                        ap=idx_all[:, t * K:(t + 1) * K], axis=0))
                return knvn

            knvn_next = gather(0)
            for t in range(T):
                r0 = t * P
                knvn = knvn_next
                if t + 1 < T:
                    knvn_next = gather(t + 1)
                kn = knvn[:].rearrange("p (j c) -> p j c", j=K)[:, :, 0:HID]
                vn = knvn[:].rearrange("p (j c) -> p j c", j=K)[:, :, HID:2 * HID]

                qb = q_all[:, t * HID:(t + 1) * HID] \
                    .rearrange("p (a c) -> p a c", a=1).to_broadcast([P, K, HID])
                prod = wp.tile([P, K * HID], F16, name="prod")
                nc.vector.tensor_tensor(
                    out=prod[:].rearrange("p (j c) -> p j c", j=K),
                    in0=kn, in1=qb, op=ALU.mult)

                # score tree over head_dim d: 16 -> 8 -> 4 -> 2 -> 1
                pv = prod[:].rearrange("p (j h d) -> p j h d", j=K, h=NH)
                sL1 = wp.tile([P, K * NH * 8], F16, name="sL1")
                nc.vector.tensor_tensor(
                    out=sL1[:].rearrange("p (j h d) -> p j h d", j=K, h=NH),
                    in0=pv[:, :, :, 0:8], in1=pv[:, :, :, 8:16], op=ALU.add)
                s1v = sL1[:].rearrange("p (j h d) -> p j h d", j=K, h=NH)
                sL2 = wp.tile([P, K * NH * 4], F16, name="sL2")
                nc.vector.tensor_tensor(
                    out=sL2[:].rearrange("p (j h d) -> p j h d", j=K, h=NH),
                    in0=s1v[:, :, :, 0:4], in1=s1v[:, :, :, 4:8], op=ALU.add)
                s2v = sL2[:].rearrange("p (j h d) -> p j h d", j=K, h=NH)
                sL3 = wp.tile([P, K * NH * 2], F16, name="sL3")
                nc.vector.tensor_tensor(
                    out=sL3[:].rearrange("p (j h d) -> p j h d", j=K, h=NH),
                    in0=s2v[:, :, :, 0:2], in1=s2v[:, :, :, 2:4], op=ALU.add)
                s3v = sL3[:].rearrange("p (j h d) -> p j h d", j=K, h=NH)
                s = wp.tile([P, K * NH], F32, name="s")
                nc.vector.tensor_tensor(
                    out=s[:].rearrange("p (j h a) -> p j h a", j=K, h=NH),
                    in0=s3v[:, :, :, 0:1], in1=s3v[:, :, :, 1:2], op=ALU.add)

                # tt = (s + 36) * mask  (on gpsimd to offload DVE)
                mb = msk_all[:, t * K:(t + 1) * K] \
                    .rearrange("p (j a) -> p j a", a=1).to_broadcast([P, K, NH])
                tt = wp.tile([P, K * NH], F32, name="tt")
                nc.gpsimd.scalar_tensor_tensor(
                    out=tt[:].rearrange("p (j h) -> p j h", j=K),
                    in0=s[:].rearrange("p (j h) -> p j h", j=K),
                    scalar=36.0, in1=mb, op0=ALU.add, op1=ALU.mult)

                # e = exp(0.25*tt - 9): compact f32 (for z) + expanded f16
                e_c = wp.tile([P, K * NH], F32, name="e_c")
                nc.scalar.activation(out=e_c[:], in_=tt[:], func=AF.Exp,
                                     bias=negq[:], scale=0.25)
                e_exp = wp.tile([P, K * HID], F16, name="e_exp")
                nc.scalar.activation(
                    out=e_exp[:].rearrange("p (j h d) -> p j h d", j=K, h=NH),
                    in_=tt[:].rearrange("p (j h) -> p j h", j=K)
                        .rearrange("p j (h a) -> p j h a", a=1)
                        .to_broadcast([P, K, NH, HD]),
                    func=AF.Exp, bias=negq[:], scale=0.25)

                # z[h] = sum_j e[j,h]
                z = wp.tile([P, NH], F32, name="z")
                nc.vector.tensor_reduce(
                    out=z[:], in_=e_c[:].rearrange("p (j h) -> p h j", j=K),
                    axis=AX.X, op=ALU.add)

                prod2 = wp.tile([P, K * HID], F16, name="prod2")
                nc.vector.tensor_tensor(
                    out=prod2[:].rearrange("p (j c) -> p j c", j=K),
                    in0=vn, in1=e_exp[:].rearrange("p (j c) -> p j c", j=K),
                    op=ALU.mult)

                # out tree over neighbors j: 16 -> 8 -> 4 -> 2 -> 1
                p2 = prod2[:].rearrange("p (j c) -> p j c", j=K)
                oL1 = wp.tile([P, 8 * HID], F16, name="oL1")
                nc.vector.tensor_tensor(
                    out=oL1[:].rearrange("p (j c) -> p j c", j=8),
                    in0=p2[:, 0:8, :], in1=p2[:, 8:16, :], op=ALU.add)
                o1 = oL1[:].rearrange("p (j c) -> p j c", j=8)
                oL2 = wp.tile([P, 4 * HID], F16, name="oL2")
                nc.vector.tensor_tensor(
                    out=oL2[:].rearrange("p (j c) -> p j c", j=4),
                    in0=o1[:, 0:4, :], in1=o1[:, 4:8, :], op=ALU.add)
                o2 = oL2[:].rearrange("p (j c) -> p j c", j=4)
                oL3 = wp.tile([P, 2 * HID], F16, name="oL3")
                nc.vector.tensor_tensor(
                    out=oL3[:].rearrange("p (j c) -> p j c", j=2),
                    in0=o2[:, 0:2, :], in1=o2[:, 2:4, :], op=ALU.add)
                o = wp.tile([P, HID], F32, name="o")
                nc.vector.tensor_tensor(
                    out=o[:], in0=oL3[:, 0:HID], in1=oL3[:, HID:2 * HID],
                    op=ALU.add)

                # out = o / z  (z broadcast over d)
                ot = wp.tile([P, HID], F32, name="ot")
                nc.vector.tensor_tensor(
                    out=ot[:].rearrange("p (h d) -> p h d", h=NH),
                    in0=o[:].rearrange("p (h d) -> p h d", h=NH),
                    in1=z[:].rearrange("p (h a) -> p h a", a=1)
                        .to_broadcast([P, NH, HD]),
                    op=ALU.divide)
                nc.sync.dma_start(out=out[r0:r0 + P, :], in_=ot[:])
    lp.__exit__(None, None, None)
    return nc


# ---------------------------------------------------------------- host side


def _prep(X, nbr_idx, nbr_mask, atom_emb, Wq, bq, Wk, bk, Wv, bv):
    X = np.asarray(X).astype(np.int64)
    emb32 = np.asarray(atom_emb, dtype=np.float32)
    # pair tables: P_f(a,b) = emb[2f][a] + emb[2f+1][b], f16 128-wide rows
    tabs = []
    for f in range(4):
        pair = (emb32[2 * f][:, None, :] + emb32[2 * f + 1][None, :, :])
        tabs.append(pair.reshape(NPAIR, HID))
    tabs.append(emb32[8])
    emb_packed = np.ascontiguousarray(
        np.concatenate(tabs, axis=0).astype(np.float16))
    assert emb_packed.shape[0] == EMB_ROWS

    # per-node 5 table-row indices
    bases = np.cumsum([0] + [NPAIR] * 4)[:5]
    xt = np.empty((X.shape[0], 5), np.int32)
    for f in range(4):
        xt[:, f] = bases[f] + X[:, 2 * f] * VOCAB + X[:, 2 * f + 1]
    xt[:, 4] = bases[4] + X[:, 8]

    g = np.asarray(nbr_idx).astype(np.int64)
    remap = ((g // NRC) * NPC + (g % NRC)).astype(np.int32)
    mask = np.asarray(nbr_mask).astype(np.uint8)
    maps = []
    for r in range(NCORES):
        lo, hi = r * NRC, (r + 1) * NRC
        xcp = np.zeros((NPC, 5), np.int32)
        xcp[:NRC] = xt[lo:hi]
        nbp = np.zeros((NPC, K), np.int32)
        nbp[:NRC] = remap[lo:hi]
        mkp = np.zeros((NPC, K), np.uint8)
        mkp[:NRC] = mask[lo:hi]
        maps.append({
            "xc": xcp, "nb": nbp, "mk": mkp, "emb": emb_packed,
            "wq": np.ascontiguousarray(np.asarray(Wq, np.float32)),
            "wk": np.ascontiguousarray(np.asarray(Wk, np.float32)),
            "wv": np.ascontiguousarray(np.asarray(Wv, np.float32)),
            "bq": np.asarray(bq, np.float32).reshape(HID, 1),
            "bk": np.asarray(bk, np.float32).reshape(HID, 1),
            "bv": np.asarray(bv, np.float32).reshape(HID, 1),
        })
    return maps


_CACHE = {}


def run_on_device(maps, trace=False):
    from concourse.bass_utils import run_bass_kernel_spmd
    if "nc" not in _CACHE:
        nc = bass.Bass()
        build(nc)
        _CACHE["nc"] = nc
    return run_bass_kernel_spmd(_CACHE["nc"], maps, list(range(NCORES)),
                                trace=trace)


def kernel(X, nbr_idx, nbr_mask, atom_emb, Wq, bq, Wk, bk, Wv, bv):
    maps = _prep(X, nbr_idx, nbr_mask, atom_emb, Wq, bq, Wk, bk, Wv, bv)
    res = run_on_device(maps)
    return np.concatenate([r["out"][:NRC] for r in res.results], axis=0)


# revision 10
# speedup vs baseline: 1.5849x; 1.0119x over previous
"""TRN2 Bass kernel for nn_GTLayer (ELL sparse attention, N=50000, K=16).

Sharding: nodes split contiguously across 8 NeuronCores (6250/core, padded
to 6272 = 49 x 128), with each core's nodes re-ordered by unmasked-neighbor
count (outputs un-permuted on the host). Per core, three phases:
  1. q|k|v via host-fused lookup tables: T_f[v] = emb_f[v] @ [Wq|Wk|Wv]
     (+biases), feature-PAIR-packed -> 5 indirect-DMA row fetches per
     128-node tile + an f16 add-tree. No matmuls on device.
  2. AllGather of kv rows across the 8 cores (index/mask loads hidden).
  3. per tile: only the J_t = max-unmasked-count neighbor rows are gathered
     (masked neighbors contribute exp(-9)~0 and are skipped entirely;
     short rows are padded with dummy slots masked to 0). Attention runs
     in f16 on DVE with binary add-trees; exp on the Act engine.
Masking: tt=(s+36)*mask, e=exp(0.25*tt-9); dummy lanes get exp(-9)~1.2e-4.
Fully-masked rows are patched exactly on the host (mean of v over the
original neighbor list).
"""
import numpy as np

import concourse.bass as bass
import concourse.mybir as mybir
import concourse.tile as tile
from concourse.vector_clock import ScopedClock

F32 = mybir.dt.float32
I32 = mybir.dt.int32
F16 = mybir.dt.float16
AX = mybir.AxisListType
ALU = mybir.AluOpType
AF = mybir.ActivationFunctionType

N_FEATS, VOCAB, HID, NH, HD, K = 9, 119, 128, 8, 16, 16
P = 128
NCORES = 8
NRC = 6250          # real nodes per core
NPC = 6272          # padded nodes per core (49 x 128)
T = NPC // P
NPAIR = VOCAB * VOCAB
# fused q|k|v lookup table: 4 pair tables + 1 single table, 384-wide f16 rows
EMB_ROWS = 4 * NPAIR + VOCAB
QKV = 3 * HID

# ---------------------------------------------------------------- walrus fixes
# This walrus build rejects >1 sync-wait command per instruction. Two fixes:
# (1) TileContext tail drain: emit waits as single-wait nops.
# (2) General: split multi-wait instructions in the serialized BIR JSON by
#     inserting single-wait NoOps immediately before them (order preserved).


def _patched_drain_and_barrier(self, tick_clock, wait_clock):
    nc = self.nc
    probe = nc.sync.nop(nofuse=True)
    wait_clock.add_sem_waits(probe.ins, ScopedClock({None: tick_clock.global_clock}))
    waits = list(probe.ins.sync_info.on_wait or []) if probe.ins.sync_info else []
    if probe.ins.sync_info:
        probe.ins.sync_info.on_wait = waits[:1]
    for w in waits[1:]:
        n2 = nc.sync.nop(nofuse=True)
        if n2.ins.sync_info is None:
            n2.ins.sync_info = mybir.SyncInfo(on_update=[], on_wait=[w])
        else:
            n2.ins.sync_info.on_wait = [w]
    nc.sync.drain()
    nc.all_engine_barrier()
    assert self.sems is not None
    popped = nc._tile_sem_poison_stack.pop()
    assert popped is self._sem_poison
    nc.clear_and_free_semaphores(list(self.sems.allocated().values()))
    nc.all_engine_barrier()


tile.TileContext._drain_and_barrier = _patched_drain_and_barrier


def _split_waits_json(bir_bytes):
    import orjson
    m = orjson.loads(bir_bytes)
    n = 0
    for fn in m["functions"]:
        for blk in fn["blocks"]:
            new = []
            for ins in blk["instructions"]:
                si = ins.get("sync_info")
                waits = (si or {}).get("on_wait") or []
                if len(waits) > 1:
                    for w in waits[:-1]:
                        n += 1
                        new.append({
                            "debug": ins.get("debug", 0),
                            "engine": ins["engine"],
                            "ins": [], "name": f"I-wfix-{n}",
                            "opcode": "NoOp", "outs": [],
                            "sync_info": {"on_update": [], "on_wait": [w]},
                        })
                    si["on_wait"] = waits[-1:]
                new.append(ins)
            blk["instructions"] = new
    return orjson.dumps(m), n


import concourse.bass2jax as _b2j

_orig_cbk = _b2j.compile_bir_kernel


def _patched_cbk(ant_bir_str, *a, **kw):
    fixed, n = _split_waits_json(ant_bir_str)
    return _orig_cbk(fixed, *a, **kw)


_b2j.compile_bir_kernel = _patched_cbk

# ---------------------------------------------------------------- device code


def build(nc, j_list, sumj):
    ntot = NPC * NCORES

    xc = nc.dram_tensor("xc", [P, T * 5], I32, kind="ExternalInput")
    nb = nc.dram_tensor("nb", [P, sumj], I32, kind="ExternalInput")
    mk = nc.dram_tensor("mk", [P, sumj], F16, kind="ExternalInput")
    emb = nc.dram_tensor("emb", [EMB_ROWS, QKV], F16, kind="ExternalInput")
    out = nc.dram_tensor("out", [NPC, HID], F32, kind="ExternalOutput")

    joff = np.concatenate([[0], np.cumsum(j_list)]).astype(int)

    lp = nc.allow_low_precision(reason="f16 attention pipeline")
    lp.__enter__()
    with tile.TileContext(nc) as tc:
        with (
            tc.tile_pool(name="const", bufs=1) as cp,
            tc.tile_pool(name="resident", bufs=1) as rp,
            tc.tile_pool(name="work", bufs=3) as wp,
            tc.tile_pool(name="gath", bufs=3) as gp,
            tc.tile_pool(name="dram", bufs=1, space="DRAM") as dp,
        ):
            negq = cp.tile([P, 1], F32, name="negq")
            nc.gpsimd.memset(negq[:], -9.0)

            xt_all = rp.tile([P, T * 5], I32, name="xt_all")
            nc.sync.dma_start(out=xt_all[:], in_=xc[:])
            idx_all = rp.tile([P, sumj], I32, name="idx_all")
            msk_all = rp.tile([P, sumj], F16, name="msk_all")
            qkv_all = rp.tile([P, T * QKV], F16, name="qkv_all")

            kv_shard = dp.tile([NPC, 2 * HID], F16, name="kv_shard")
            kv_full = dp.tile([ntot, 2 * HID], F16, name="kv_full",
                              addr_space="Shared")

            # ---------------- phase 1: fused q|k|v lookup + add-tree
            for t in range(T):
                r0 = t * P
                et = gp.tile([P, 5 * QKV], F16, name="et")
                for c in range(5):
                    nc.gpsimd.indirect_dma_start(
                        out=et[:, c * QKV:(c + 1) * QKV], out_offset=None,
                        in_=emb[:],
                        in_offset=bass.IndirectOffsetOnAxis(
                            ap=xt_all[:, t * 5 + c:t * 5 + c + 1], axis=0))
                eL1 = wp.tile([P, 2 * QKV], F16, name="eL1")
                nc.vector.tensor_tensor(
                    out=eL1[:], in0=et[:, 0:2 * QKV],
                    in1=et[:, 2 * QKV:4 * QKV], op=ALU.add)
                eL2 = wp.tile([P, QKV], F16, name="eL2")
                nc.vector.tensor_tensor(
                    out=eL2[:], in0=eL1[:, 0:QKV], in1=eL1[:, QKV:2 * QKV],
                    op=ALU.add)
                nc.vector.tensor_tensor(
                    out=qkv_all[:, t * QKV:(t + 1) * QKV], in0=eL2[:],
                    in1=et[:, 4 * QKV:5 * QKV], op=ALU.add)
                nc.sync.dma_start(
                    out=kv_shard[r0:r0 + P, :],
                    in_=qkv_all[:, t * QKV + HID:(t + 1) * QKV])

            # ---------------- phase 2: allgather kv; loads hidden under it
            nc.gpsimd.collective_compute(
                "AllGather", ALU.bypass,
                replica_groups=[list(range(NCORES))],
                ins=[kv_shard[:]], outs=[kv_full[:]])

            nc.sync.dma_start(out=idx_all[:], in_=nb[:])
            nc.sync.dma_start(out=msk_all[:], in_=mk[:])

            # ---------------- phase 3: J_t neighbor gathers + attention
            def gather(t):
                J = j_list[t]
                knvn = gp.tile([P, K * 2 * HID], F16, name="knvn")
                for j in range(J):
                    o = joff[t] + j
                    nc.gpsimd.indirect_dma_start(
                        out=knvn[:, j * 2 * HID:(j + 1) * 2 * HID],
                        out_offset=None, in_=kv_full[:],
                        in_offset=bass.IndirectOffsetOnAxis(
                            ap=idx_all[:, o:o + 1], axis=0))
                return knvn

            knvn_next = gather(0)
            for t in range(T):
                r0 = t * P
                J = j_list[t]
                knvn = knvn_next
                if t + 1 < T:
                    knvn_next = gather(t + 1)
                kview = knvn[:].rearrange("p (j c) -> p j c", j=K)
                kn = kview[:, 0:J, 0:HID]
                vn = kview[:, 0:J, HID:2 * HID]

                qb = qkv_all[:, t * QKV:t * QKV + HID] \
                    .rearrange("p (a c) -> p a c", a=1).to_broadcast([P, J, HID])
                prod = wp.tile([P, K * HID], F16, name="prod")
                nc.vector.tensor_tensor(
                    out=prod[:, 0:J * HID].rearrange("p (j c) -> p j c", j=J),
                    in0=kn, in1=qb, op=ALU.mult)

                # score tree over head_dim d: 16 -> 8 -> 4 -> 2 -> 1
                pv = prod[:, 0:J * HID].rearrange(
                    "p (j h d) -> p j h d", j=J, h=NH)
                sL1 = wp.tile([P, K * NH * 8], F16, name="sL1")
                nc.vector.tensor_tensor(
                    out=sL1[:, 0:J * NH * 8].rearrange(
                        "p (j h d) -> p j h d", j=J, h=NH),
                    in0=pv[:, :, :, 0:8], in1=pv[:, :, :, 8:16], op=ALU.add)
                s1v = sL1[:, 0:J * NH * 8].rearrange(
                    "p (j h d) -> p j h d", j=J, h=NH)
                sL2 = wp.tile([P, K * NH * 4], F16, name="sL2")
                nc.vector.tensor_tensor(
                    out=sL2[:, 0:J * NH * 4].rearrange(
                        "p (j h d) -> p j h d", j=J, h=NH),
                    in0=s1v[:, :, :, 0:4], in1=s1v[:, :, :, 4:8], op=ALU.add)
                s2v = sL2[:, 0:J * NH * 4].rearrange(
                    "p (j h d) -> p j h d", j=J, h=NH)
                sL3 = wp.tile([P, K * NH * 2], F16, name="sL3")
                nc.vector.tensor_tensor(
                    out=sL3[:, 0:J * NH * 2].rearrange(
                        "p (j h d) -> p j h d", j=J, h=NH),
                    in0=s2v[:, :, :, 0:2], in1=s2v[:, :, :, 2:4], op=ALU.add)
                s3v = sL3[:, 0:J * NH * 2].rearrange(
                    "p (j h d) -> p j h d", j=J, h=NH)
                s = wp.tile([P, K * NH], F32, name="s")
                nc.vector.tensor_tensor(
                    out=s[:, 0:J * NH].rearrange(
                        "p (j h a) -> p j h a", j=J, h=NH),
                    in0=s3v[:, :, :, 0:1], in1=s3v[:, :, :, 1:2], op=ALU.add)

                # tt = (s + 36) * mask
                mb = msk_all[:, joff[t]:joff[t] + J] \
                    .rearrange("p (j a) -> p j a", a=1).to_broadcast([P, J, NH])
                tt = wp.tile([P, K * NH], F32, name="tt")
                nc.vector.scalar_tensor_tensor(
                    out=tt[:, 0:J * NH].rearrange("p (j h) -> p j h", j=J),
                    in0=s[:, 0:J * NH].rearrange("p (j h) -> p j h", j=J),
                    scalar=36.0, in1=mb, op0=ALU.add, op1=ALU.mult)

                # e = exp(0.25*tt - 9): compact f32 (for z) + expanded f16
                e_c = wp.tile([P, K * NH], F32, name="e_c")
                nc.scalar.activation(out=e_c[:, 0:J * NH], in_=tt[:, 0:J * NH],
                                     func=AF.Exp, bias=negq[:], scale=0.25)
                e_exp = wp.tile([P, K * HID], F16, name="e_exp")
                nc.scalar.activation(
                    out=e_exp[:, 0:J * HID].rearrange(
                        "p (j h d) -> p j h d", j=J, h=NH),
                    in_=tt[:, 0:J * NH].rearrange("p (j h) -> p j h", j=J)
                        .rearrange("p j (h a) -> p j h a", a=1)
                        .to_broadcast([P, J, NH, HD]),
                    func=AF.Exp, bias=negq[:], scale=0.25)

                # z[h] = sum_j e[j,h]
                z = wp.tile([P, NH], F32, name="z")
                nc.vector.tensor_reduce(
                    out=z[:],
                    in_=e_c[:, 0:J * NH].rearrange("p (j h) -> p h j", j=J),
                    axis=AX.X, op=ALU.add)
                zr = wp.tile([P, NH], F32, name="zr")
                nc.vector.reciprocal(out=zr[:], in_=z[:])

                prod2 = wp.tile([P, K * HID], F16, name="prod2")
                nc.vector.tensor_tensor(
                    out=prod2[:, 0:J * HID].rearrange("p (j c) -> p j c", j=J),
                    in0=vn,
                    in1=e_exp[:, 0:J * HID].rearrange("p (j c) -> p j c", j=J),
                    op=ALU.mult)

                # out tree over neighbors j (generic halving with odd carry)
                o = wp.tile([P, HID], F32, name="o")
                if J == 1:
                    nc.vector.tensor_copy(out=o[:], in_=prod2[:, 0:HID])
                else:
                    cur, curJ, lvl = prod2, J, 0
                    while curJ > 1:
                        half = curJ // 2
                        odd = curJ - 2 * half
                        nsz = half + odd
                        if nsz == 1:
                            nc.vector.tensor_tensor(
                                out=o[:], in0=cur[:, 0:HID],
                                in1=cur[:, HID:2 * HID], op=ALU.add)
                        else:
                            nt = wp.tile([P, 8 * HID], F16, name=f"oT{lvl}")
                            nc.vector.tensor_tensor(
                                out=nt[:, 0:half * HID],
                                in0=cur[:, 0:half * HID],
                                in1=cur[:, half * HID:2 * half * HID],
                                op=ALU.add)
                            if odd:
                                nc.vector.tensor_copy(
                                    out=nt[:, half * HID:nsz * HID],
                                    in_=cur[:, 2 * half * HID:curJ * HID])
                            cur = nt
                        curJ = nsz
                        lvl += 1

                ot = wp.tile([P, HID], F32, name="ot")
                nc.vector.tensor_tensor(
                    out=ot[:].rearrange("p (h d) -> p h d", h=NH),
                    in0=o[:].rearrange("p (h d) -> p h d", h=NH),
                    in1=zr[:].rearrange("p (h a) -> p h a", a=1)
                        .to_broadcast([P, NH, HD]),
                    op=ALU.mult)
                nc.sync.dma_start(out=out[r0:r0 + P, :], in_=ot[:])
    lp.__exit__(None, None, None)
    return nc


# ---------------------------------------------------------------- host side


def _prep(X, nbr_idx, nbr_mask, atom_emb, Wq, bq, Wk, bk, Wv, bv):
    X = np.asarray(X).astype(np.int64)
    N = X.shape[0]
    emb32 = np.asarray(atom_emb, dtype=np.float32)
    Wq = np.asarray(Wq, np.float32)
    Wk = np.asarray(Wk, np.float32)
    Wv = np.asarray(Wv, np.float32)
    bq = np.asarray(bq, np.float32).reshape(-1)
    bk = np.asarray(bk, np.float32).reshape(-1)
    bv = np.asarray(bv, np.float32).reshape(-1)

    # fused per-feature q|k|v tables; pair-packed; biases folded into table 8
    eq = emb32 @ Wq          # [9, VOCAB, HID]
    ek = emb32 @ Wk
    ev = emb32 @ Wv
    fused = np.concatenate([eq, ek, ev], axis=2)     # [9, VOCAB, 3H]
    tabs = []
    for f in range(4):
        pair = fused[2 * f][:, None, :] + fused[2 * f + 1][None, :, :]
        tabs.append(pair.reshape(NPAIR, QKV))
    t8 = fused[8] + np.concatenate([bq, bk, bv])[None, :]
    tabs.append(t8)
    emb_packed = np.ascontiguousarray(
        np.concatenate(tabs, axis=0).astype(np.float16))
    assert emb_packed.shape[0] == EMB_ROWS

    bases = np.array([0, NPAIR, 2 * NPAIR, 3 * NPAIR, 4 * NPAIR])
    xt = np.empty((N, 5), np.int32)
    for f in range(4):
        xt[:, f] = bases[f] + X[:, 2 * f] * VOCAB + X[:, 2 * f + 1]
    xt[:, 4] = bases[4] + X[:, 8]

    g = np.asarray(nbr_idx).astype(np.int64)
    mask = np.asarray(nbr_mask).astype(bool)
    u = mask.sum(axis=1)                             # unmasked counts

    # per-core node order: sort by unmasked count (stable)
    perms, inv_perms = [], []
    for r in range(NCORES):
        lo = r * NRC
        p = np.argsort(u[lo:lo + NRC], kind="stable")
        perms.append(p + lo)                         # global ids, sorted
        ip = np.empty(NRC, np.int64)
        ip[p] = np.arange(NRC)
        inv_perms.append(ip)

    # kv_full row of global node n (after per-core permutation)
    row_of = np.empty(N, np.int64)
    for r in range(NCORES):
        row_of[perms[r]] = r * NPC + np.arange(NRC)

    # per-core per-tile J (max unmasked in tile), then cross-core max
    j_tiles = np.zeros((NCORES, T), np.int64)
    for r in range(NCORES):
        uu = np.zeros(NPC, np.int64)
        uu[:NRC] = u[perms[r]]
        j_tiles[r] = uu.reshape(T, P).max(axis=1)
    j_list = np.maximum(j_tiles.max(axis=0), 1).astype(int)
    joff = np.concatenate([[0], np.cumsum(j_list)]).astype(int)
    sumj = int(j_list.sum())

    maps = []
    for r in range(NCORES):
        ids = perms[r]                               # sorted global node ids
        xcp = np.zeros((NPC, 5), np.int32)
        xcp[:NRC] = xt[ids]
        # pack unmasked neighbors first, per node; dummies are row 0 / mask 0
        nbp = np.zeros((P, sumj), np.int32)
        mkp = np.zeros((P, sumj), np.float16)
        rows_r = row_of[g[ids]]                      # [NRC, K] kv_full rows
        msk_r = mask[ids]                            # [NRC, K]
        for t in range(T):
            J = int(j_list[t])
            base = int(joff[t])
            for pp in range(P):
                i = t * P + pp
                if i >= NRC:
                    continue
                sel = rows_r[i][msk_r[i]]
                nslot = min(len(sel), J)
                nbp[pp, base:base + nslot] = sel[:nslot]
                mkp[pp, base:base + nslot] = 1.0
        maps.append({
            "xc": np.ascontiguousarray(
                xcp.reshape(T, P, 5).transpose(1, 0, 2).reshape(P, T * 5)),
            "nb": nbp, "mk": mkp, "emb": emb_packed,
        })

    # exact host patch for fully-masked rows: uniform average of v over the
    # ORIGINAL neighbor list (matches jax softmax of all -1e9)
    patches = []
    zrows = np.nonzero(u == 0)[0]
    if len(zrows):
        for n in zrows:
            nbrs = g[n]
            h_n = emb32[np.arange(N_FEATS)[None, :], X[nbrs]].sum(1)
            v_n = h_n @ Wv + bv[None, :]
            patches.append((int(n), v_n.mean(axis=0)))

    meta = {"j_list": [int(j) for j in j_list], "sumj": sumj, "perms": perms,
            "inv_perms": inv_perms, "patches": patches}
    return maps, meta


_CACHE = {}


def run_on_device(maps, meta, trace=False):
    from concourse.bass_utils import run_bass_kernel_spmd
    key = (tuple(meta["j_list"]), meta["sumj"])
    if _CACHE.get("key") != key:
        nc = bass.Bass()
        build(nc, meta["j_list"], meta["sumj"])
        _CACHE["nc"] = nc
        _CACHE["key"] = key
    return run_bass_kernel_spmd(_CACHE["nc"], maps, list(range(NCORES)),
                                trace=trace)


def kernel(X, nbr_idx, nbr_mask, atom_emb, Wq, bq, Wk, bk, Wv, bv):
    maps, meta = _prep(X, nbr_idx, nbr_mask, atom_emb, Wq, bq, Wk, bk, Wv, bv)
    res = run_on_device(maps, meta)
    outs = []
    for r in range(NCORES):
        o = res.results[r]["out"][:NRC]
        outs.append(o[meta["inv_perms"][r]])         # undo per-core sort
    full = np.concatenate(outs, axis=0)
    for n, v in meta["patches"]:
        full[n] = v
    return full
